# revision 28
# baseline (speedup 1.0000x reference)
"""Multi-head attention (B=2, S=2048, D=1024, H=16) on 8 Trainium2 NeuronCores.

Sharding: core c handles (batch b=c//4, query chunk j=c%4 of 512 rows).
 - Each core computes K^T / V for its WHOLE batch locally (weights replicated,
   pre-transposed + bf16-cast on host; softmax scale folded into W_q) — no
   collectives, the PE stays continuously busy so the HAM clock-gate stays
   warm.
 - Q projected for the core's own 512 rows only.
 - Attention (all 16 heads, 512 queries x 2048 keys):
   scoresT = K_h @ Q_h^T  ->  exp on ACT  ->  attnT = [V_h|1]^T @ E
   (ones column gives the softmax denominator Z in row 64 of attnT psum).
 - Q/K biases folded into the projection casts (ACT Identity per-partition
   bias); V bias folded into the output bias on host (sum(probs) == 1).
 - Output projection local per 512-row chunk; final output assembled on host.
"""

import numpy as np
import ml_dtypes

import concourse.bass as bass
import concourse.mybir as mybir
import concourse.tile as tile
from concourse import bacc
from concourse.bass_utils import run_bass_kernel_spmd

BF16 = mybir.dt.bfloat16
F32 = mybir.dt.float32
AF = mybir.ActivationFunctionType

B, S, D = 2, 2048, 1024
H, HD = 16, 64
N_CORES = 8
R = 4            # cores per batch
SL = S // R      # local query rows per core (512)
P = 128
DCH = D // P     # 8 d-chunks
NKK = S // P     # 16 key chunks
ET = D // P      # 8 feature tiles per projection
FREE = 512


def build_program():
    nc = bacc.Bacc("TRN2", target_bir_lowering=False, debug=False,
                   num_devices=N_CORES)

    xT = nc.dram_tensor("xT", [D, S], BF16, kind="ExternalInput")
    xqT = nc.dram_tensor("xqT", [D, SL], BF16, kind="ExternalInput")
    wqkvT = nc.dram_tensor("wqkvT", [D, 3 * D], BF16, kind="ExternalInput")
    bqk = nc.dram_tensor("bqk", [P, 16], BF16, kind="ExternalInput")
    woutT = nc.dram_tensor("woutT", [D, D], BF16, kind="ExternalInput")
    bout = nc.dram_tensor("bout", [1, D], BF16, kind="ExternalInput")
    out = nc.dram_tensor("out", [SL, D], F32, kind="ExternalOutput")

    with tile.TileContext(nc) as tc:
        _build(nc, tc, xT, xqT, wqkvT, bqk, woutT, bout, out)
    nc.compile()
    return nc


def _build(nc, tc, xT, xqT, wqkvT, bqk, woutT, bout, out):
    from contextlib import ExitStack

    ctx = ExitStack()
    consts = ctx.enter_context(tc.tile_pool(name="consts", bufs=1))

    # ---- constants ----
    ones_bf = consts.tile([1, FREE], BF16, name="ones_bf")
    nc.vector.memset(ones_bf[:], 1.0)
    bqk_sb = consts.tile([P, 16], BF16, name="bqk_sb")
    nc.sync.dma_start(bqk_sb[:], bqk.ap())
    bout_sb = consts.tile([1, D], BF16, name="bout_sb")
    nc.sync.dma_start(bout_sb[:], bout.ap())

    # ---- resident input tiles ----
    xt_pool = ctx.enter_context(tc.tile_pool(name="xt", bufs=1))
    xt = []
    for i in range(DCH):
        t = xt_pool.tile([P, S], BF16, name=f"xt{i}")
        for ch in range(4):
            nc.sync.dma_start(t[:, FREE * ch:FREE * (ch + 1)],
                              xT.ap()[P * i:P * (i + 1),
                                      FREE * ch:FREE * (ch + 1)])
        xt.append(t)
    xq = []
    for i in range(DCH):
        t = xt_pool.tile([P, SL], BF16, name=f"xq{i}")
        nc.sync.dma_start(t[:], xqT.ap()[P * i:P * (i + 1), :])
        xq.append(t)

    # ---- weight stream (K, V blocks first, then Q) ----
    w_pool = ctx.enter_context(tc.tile_pool(name="wq", bufs=16))
    wblk = {}

    def load_w(ebs):
        for eb in ebs:
            for d in range(DCH):
                t = w_pool.tile([P, FREE], BF16, name=f"w{eb}_{d}", tag="w")
                nc.gpsimd.dma_start(t[:], wqkvT.ap()[P * d:P * (d + 1),
                                                     FREE * eb:FREE * (eb + 1)])
                wblk[(eb, d)] = t

    # V layout: per key-tile, 16 heads x (64 V-features + ones col) packed at
    # stride 65, plus 64 zero columns of tail pad so a 128-wide stationary
    # slice [65h : 65h+128] is always in bounds. Columns 65..127 of that
    # slice hit neighbor-head data; they only feed psum rows 65..127 which
    # are never read. Full 128x128 stationary keeps the PE clock-gate warm.
    VW = H * (HD + 1) + HD  # 1104
    kv_pool = ctx.enter_context(tc.tile_pool(name="kv", bufs=1))
    kt = [kv_pool.tile([P, S], BF16, name=f"kt{t}") for t in range(ET)]
    vt = [kv_pool.tile([P, VW], BF16, name=f"vt{g}") for g in range(NKK)]
    for g in range(NKK):
        v3 = vt[g][:, 0:H * (HD + 1)].rearrange("p (h c) -> p h c", c=HD + 1)
        nc.vector.memset(v3[:, :, HD:HD + 1], 1.0)
        nc.vector.memset(vt[g][:, H * (HD + 1):VW], 0.0)
    # Q, zero-padded per head: head h occupies partitions 64*(h%2)..+64 of
    # qz[h], the other 64 partitions are zero -> scores matmul can use the
    # full 128-partition K^T pair tile as stationary (K=128, stays warm).
    qt_pool = ctx.enter_context(tc.tile_pool(name="qt", bufs=1))
    qz = [qt_pool.tile([P, FREE], BF16, name=f"qz{h}") for h in range(H)]
    for h in range(H):
        off = HD * ((h + 1) % 2)
        nc.vector.memset(qz[h][off:off + HD, :], 0.0)

    # ---- K^T projection, full batch: out[e, s] ----
    load_w((2, 3))
    with tc.tile_pool(name="projk_ps", bufs=8, space="PSUM") as ps_pool:
        for t in range(ET):
            eb = 2 + t // 4
            co = P * (t % 4)
            pss = [ps_pool.tile([P, FREE], F32, name=f"psk{t}_{sch}",
                                tag="proj") for sch in range(4)]
            # d-loop outer: 4 consecutive matmuls share one stationary tile
            for d in range(DCH):
                for sch in range(4):
                    nc.tensor.matmul(pss[sch][:],
                                     wblk[(eb, d)][:, co:co + P],
                                     xt[d][:, FREE * sch:FREE * (sch + 1)],
                                     start=(d == 0), stop=(d == DCH - 1))
            for sch in range(4):
                # cast + K-bias (per-partition) fused on ACT
                nc.scalar.activation(kt[t][:, FREE * sch:FREE * (sch + 1)],
                                     pss[sch][:], AF.Identity,
                                     bias=bqk_sb[:, 8 + t:9 + t])

    # ---- V projection, full batch, natural: out[s, e] ----
    load_w((4, 5))
    with tc.tile_pool(name="projv_ps", bufs=8, space="PSUM") as ps_pool:
        for st in range(NKK):
            pss = [ps_pool.tile([P, FREE], F32, name=f"psv{st}_{eb}",
                                tag="proj") for eb in range(2)]
            for d in range(DCH):
                for eb in range(2):
                    nc.tensor.matmul(pss[eb][:],
                                     xt[d][:, P * st:P * (st + 1)],
                                     wblk[(4 + eb, d)][:],
                                     start=(d == 0), stop=(d == DCH - 1))
            for eb in range(2):
                # cast to the [V|1] attention layout on DVE (no bias:
                # V-bias is folded into the output bias on host)
                v3 = vt[st][:, 0:H * (HD + 1)].rearrange(
                    "p (h c) -> p h c", c=HD + 1)
                nc.vector.tensor_copy(
                    v3[:, 8 * eb:8 * (eb + 1), 0:HD],
                    pss[eb].rearrange("p (h d) -> p h d", d=HD))

    # ---- Q projection (own 512 rows): out[e, q] ----
    load_w((0, 1))
    with tc.tile_pool(name="projq_ps", bufs=4, space="PSUM") as ps_pool:
        for t in range(ET):
            eb = t // 4
            co = P * (t % 4)
            ps = ps_pool.tile([P, FREE], F32, name=f"psq{t}", tag="proj")
            for d in range(DCH):
                nc.tensor.matmul(ps[:], wblk[(eb, d)][:, co:co + P], xq[d][:],
                                 start=(d == 0), stop=(d == DCH - 1))
            nc.scalar.activation(qz[2 * t][0:HD, :], ps[0:HD, :], AF.Identity,
                                 bias=bqk_sb[0:HD, t:t + 1])
            nc.scalar.activation(qz[2 * t + 1][HD:P, :], ps[HD:P, :],
                                 AF.Identity, bias=bqk_sb[HD:P, t:t + 1])

    # ---- prefetch output-projection weights ----
    wo_pool = ctx.enter_context(tc.tile_pool(name="wo", bufs=1))
    wo = []
    for p_ in range(DCH):
        t = wo_pool.tile([P, D], BF16, name=f"wo{p_}")
        nc.sync.dma_start(t[:], woutT.ap()[P * p_:P * (p_ + 1), :])
        wo.append(t)

    # ---- attention, with the output projection interleaved ----
    # outproj partials accumulate into SBUF f32 via DVE, so only one extra
    # PSUM bank is needed and the outproj hides inside attention's ACT slack.
    attn_sb_pool = ctx.enter_context(tc.tile_pool(name="attnsb", bufs=1))
    small_pool = ctx.enter_context(tc.tile_pool(name="small", bufs=4))
    osb_pool = ctx.enter_context(tc.tile_pool(name="osb", bufs=1))
    attn_sb = [attn_sb_pool.tile([P, FREE], BF16, name=f"attnsb{p_}")
               for p_ in range(H // 2)]
    osb = [osb_pool.tile([P, D], F32, name=f"osb{st}")
           for st in range(SL // P)]
    GRP = 2  # kk-chunks per score-psum tile
    with tc.tile_pool(name="sc_ps", bufs=2, space="PSUM") as sc_ps, \
         tc.tile_pool(name="atbc_ps", bufs=3, space="PSUM") as atbc_ps, \
         tc.tile_pool(name="op_ps", bufs=1, space="PSUM") as op_ps, \
         tc.tile_pool(name="e_sb", bufs=4) as e_pool:
        # init osb with the output bias (broadcast via PE outer-product)
        for st in range(SL // P):
            for eb in range(2):
                bi = op_ps.tile([P, FREE], F32, name=f"bi{st}_{eb}", tag="op")
                nc.tensor.matmul(bi[:], ones_bf[:, :P],
                                 bout_sb[:, FREE * eb:FREE * (eb + 1)],
                                 start=True, stop=True)
                nc.vector.tensor_copy(osb[st][:, FREE * eb:FREE * (eb + 1)],
                                      bi[:])

        pending = []
        pv_pending = []
        op_tasks = []

        def normalize(h, at):
            # attn = attnT[0:64] * (1/Z) ; Z = attnT row 64.
            # Broadcast Z first (PE outer-product waits only on the cheap
            # Z-row copy), then reciprocal on DVE off the PE critical path.
            koff = HD * (h % 2)
            zr = small_pool.tile([1, FREE], BF16, name=f"zr{h}", tag="zr")
            nc.vector.tensor_copy(zr[:], at[HD:HD + 1, :])
            bc = atbc_ps.tile([HD, FREE], F32, name=f"bc{h}", tag="atbc")
            nc.tensor.matmul(bc[:], ones_bf[:, :HD], zr[:],
                             start=True, stop=True)
            rzb = small_pool.tile([HD, FREE], F32, name=f"rzb{h}", tag="rzb")
            nc.vector.reciprocal(rzb[:], bc[:])
            nc.vector.tensor_mul(attn_sb[h // 2][koff:koff + HD, :],
                                 at[0:HD, :], rzb[:])
            # pair h//2 fully normalized once h is odd -> queue its outproj
            if h % 2 == 1:
                p_ = h // 2
                for st in range(SL // P):
                    for eb in range(2):
                        op_tasks.append((p_, st, eb))

        def run_op(p_, st, eb):
            op = op_ps.tile([P, FREE], F32, name=f"op{p_}_{st}_{eb}",
                            tag="op")
            nc.tensor.matmul(op[:], attn_sb[p_][:, P * st:P * (st + 1)],
                             wo[p_][:, FREE * eb:FREE * (eb + 1)],
                             start=True, stop=True)
            nc.vector.tensor_add(osb[st][:, FREE * eb:FREE * (eb + 1)],
                                 osb[st][:, FREE * eb:FREE * (eb + 1)],
                                 op[:])

        def attn_v(h, at, g, e):
            for j in range(GRP):
                kk = GRP * g + j
                nc.tensor.matmul(at[:], vt[kk][:, 65 * h:65 * h + P],
                                 e[:, FREE * j:FREE * (j + 1)],
                                 start=(kk == 0), stop=(kk == NKK - 1))

        for h in range(H):
            ktile = h // 2
            q_rhs = qz[h][:]
            at = atbc_ps.tile([P, FREE], F32, name=f"at{h}", tag="atbc")
            for g in range(NKK // GRP):
                sc = sc_ps.tile([P, GRP * FREE], F32, name=f"sc{h}_{g}",
                                tag="sc")
                for j in range(GRP):
                    kk = GRP * g + j
                    nc.tensor.matmul(
                        sc[:, FREE * j:FREE * (j + 1)],
                        kt[ktile][:, P * kk:P * (kk + 1)],
                        q_rhs, start=True, stop=True)
                e = e_pool.tile([P, GRP * FREE], BF16, name=f"e{h}_{g}",
                                tag="e")
                nc.scalar.activation(e[:], sc[:], AF.Exp)
                # run the PV matmuls one group behind the scores stream, so
                # the PE never waits on the exp it just produced
                if pv_pending:
                    attn_v(*pv_pending.pop())
                pv_pending.append((h, at, g, e))
                if g == 2 and pending:
                    normalize(*pending.pop())
                if op_tasks:
                    run_op(*op_tasks.pop(0))
            pending.append((h, at))
        attn_v(*pv_pending.pop())
        normalize(*pending.pop())
        while op_tasks:
            run_op(*op_tasks.pop(0))
        for st in range(SL // P):
            nc.sync.dma_start(out.ap()[P * st:P * (st + 1), :], osb[st][:])

    ctx.close()


_CACHE = {}


def _get_program():
    if "nc" not in _CACHE:
        _CACHE["nc"] = build_program()
    return _CACHE["nc"]


def prep_inputs(input_tensor, qkv_weight, qkv_bias, out_weight, out_bias):
    """Host-side shard + transpose + cast. Returns in_maps for 8 cores."""
    x = np.asarray(input_tensor, np.float32)
    wqkv = np.asarray(qkv_weight, np.float32).copy()
    bq = np.asarray(qkv_bias, np.float32).copy()
    wout = np.asarray(out_weight, np.float32)
    scale = 1.0 / np.sqrt(np.float32(HD))
    wqkv[:D] *= scale
    bq[:D] *= scale
    bf = ml_dtypes.bfloat16
    wqkvT = np.ascontiguousarray(wqkv.T).astype(bf)
    # Q/K biases, column-major per 128-feature tile: bqk[:, t] = bias tile t
    bqk = np.ascontiguousarray(bq[:2 * D].reshape(16, P).T).astype(bf)
    woutT = np.ascontiguousarray(wout.T).astype(bf)
    # V bias folded into output bias: probs @ (V + b_v) = probs @ V + b_v
    bout_eff = np.asarray(out_bias, np.float32) + wout @ bq[2 * D:]
    bout = bout_eff.reshape(1, D).astype(bf)
    xTb = [np.ascontiguousarray(x[b].T).astype(bf) for b in range(B)]
    in_maps = []
    for c in range(N_CORES):
        b, j = c // R, c % R
        xqT = np.ascontiguousarray(xTb[b][:, SL * j:SL * (j + 1)])
        in_maps.append({"xT": xTb[b], "xqT": xqT, "wqkvT": wqkvT,
                        "bqk": bqk, "woutT": woutT, "bout": bout})
    return in_maps


def kernel(input_tensor, qkv_weight, qkv_bias, out_weight, out_bias,
           **run_kwargs):
    nc = _get_program()
    in_maps = prep_inputs(input_tensor, qkv_weight, qkv_bias, out_weight,
                          out_bias)
    res = run_bass_kernel_spmd(nc, in_maps, core_ids=list(range(N_CORES)),
                               **run_kwargs)
    full = np.empty((B, S, D), np.float32)
    for c in range(N_CORES):
        b, j = c // R, c % R
        full[b, SL * j:SL * (j + 1), :] = res.results[c]["out"]
    if run_kwargs:
        kernel.last_results = res
    return full


# revision 29
# speedup vs baseline: 1.0642x; 1.0642x over previous
"""Multi-head attention (B=2, S=2048, D=1024, H=16) on 8 Trainium2 NeuronCores.

Sharding: core c handles (batch b=c//4, query chunk j=c%4 of 512 rows).
 - Each core computes K^T / V for its WHOLE batch locally (weights replicated,
   pre-transposed + bf16-cast on host; softmax scale folded into W_q) — no
   collectives, the PE stays continuously busy so the HAM clock-gate stays
   warm.
 - Q projected for the core's own 512 rows only.
 - Attention (all 16 heads, 512 queries x 2048 keys):
   scoresT = K_h @ Q_h^T  ->  exp on ACT  ->  attnT = [V_h|1]^T @ E
   (ones column gives the softmax denominator Z in row 64 of attnT psum).
 - Q/K biases folded into the projection casts (ACT Identity per-partition
   bias); V bias folded into the output bias on host (sum(probs) == 1).
 - Output projection local per 512-row chunk; final output assembled on host.
"""

import numpy as np
import ml_dtypes

import concourse.bass as bass
import concourse.mybir as mybir
import concourse.tile as tile
from concourse import bacc
from concourse.bass_utils import run_bass_kernel_spmd

BF16 = mybir.dt.bfloat16
F32 = mybir.dt.float32
AF = mybir.ActivationFunctionType

B, S, D = 2, 2048, 1024
H, HD = 16, 64
N_CORES = 8
R = 4            # cores per batch
SL = S // R      # local query rows per core (512)
P = 128
DCH = D // P     # 8 d-chunks
NKK = S // P     # 16 key chunks
ET = D // P      # 8 feature tiles per projection
FREE = 512


def build_program():
    nc = bacc.Bacc("TRN2", target_bir_lowering=False, debug=False,
                   num_devices=N_CORES)

    xT = nc.dram_tensor("xT", [D, S], BF16, kind="ExternalInput")
    xqT = nc.dram_tensor("xqT", [D, SL], BF16, kind="ExternalInput")
    wqkvT = nc.dram_tensor("wqkvT", [D, 3 * D], BF16, kind="ExternalInput")
    bqk = nc.dram_tensor("bqk", [P, 16], BF16, kind="ExternalInput")
    woutT = nc.dram_tensor("woutT", [D, D], BF16, kind="ExternalInput")
    bout = nc.dram_tensor("bout", [1, D], BF16, kind="ExternalInput")
    out = nc.dram_tensor("out", [SL, D], F32, kind="ExternalOutput")

    with tile.TileContext(nc) as tc:
        _build(nc, tc, xT, xqT, wqkvT, bqk, woutT, bout, out)
    nc.compile()
    return nc


def _build(nc, tc, xT, xqT, wqkvT, bqk, woutT, bout, out):
    from contextlib import ExitStack

    ctx = ExitStack()
    consts = ctx.enter_context(tc.tile_pool(name="consts", bufs=1))

    # ---- constants ----
    ones_bf = consts.tile([1, FREE], BF16, name="ones_bf")
    nc.vector.memset(ones_bf[:], 1.0)
    bqk_sb = consts.tile([P, 16], BF16, name="bqk_sb")
    nc.sync.dma_start(bqk_sb[:], bqk.ap())
    bout_sb = consts.tile([1, D], BF16, name="bout_sb")
    nc.sync.dma_start(bout_sb[:], bout.ap())

    # ---- resident input tiles ----
    xt_pool = ctx.enter_context(tc.tile_pool(name="xt", bufs=1))
    xt = []
    for i in range(DCH):
        t = xt_pool.tile([P, S], BF16, name=f"xt{i}")
        for ch in range(4):
            nc.sync.dma_start(t[:, FREE * ch:FREE * (ch + 1)],
                              xT.ap()[P * i:P * (i + 1),
                                      FREE * ch:FREE * (ch + 1)])
        xt.append(t)
    xq = []
    for i in range(DCH):
        t = xt_pool.tile([P, SL], BF16, name=f"xq{i}")
        nc.sync.dma_start(t[:], xqT.ap()[P * i:P * (i + 1), :])
        xq.append(t)

    # ---- weight stream (K, V blocks first, then Q) ----
    w_pool = ctx.enter_context(tc.tile_pool(name="wq", bufs=16))
    wblk = {}

    def load_w(ebs):
        for eb in ebs:
            for d in range(DCH):
                t = w_pool.tile([P, FREE], BF16, name=f"w{eb}_{d}", tag="w")
                nc.gpsimd.dma_start(t[:], wqkvT.ap()[P * d:P * (d + 1),
                                                     FREE * eb:FREE * (eb + 1)])
                wblk[(eb, d)] = t

    # V layout: per key-tile, 16 heads x (64 V-features + ones col) packed at
    # stride 65, plus 64 zero columns of tail pad so a 128-wide stationary
    # slice [65h : 65h+128] is always in bounds. Columns 65..127 of that
    # slice hit neighbor-head data; they only feed psum rows 65..127 which
    # are never read. Full 128x128 stationary keeps the PE clock-gate warm.
    VW = H * (HD + 1) + HD  # 1104
    kv_pool = ctx.enter_context(tc.tile_pool(name="kv", bufs=1))
    kt = [kv_pool.tile([P, S], BF16, name=f"kt{t}") for t in range(ET)]
    vt = [kv_pool.tile([P, VW], BF16, name=f"vt{g}") for g in range(NKK)]
    for g in range(NKK):
        v3 = vt[g][:, 0:H * (HD + 1)].rearrange("p (h c) -> p h c", c=HD + 1)
        nc.vector.memset(v3[:, :, HD:HD + 1], 1.0)
        nc.vector.memset(vt[g][:, H * (HD + 1):VW], 0.0)
    # Q, zero-padded per head: head h occupies partitions 64*(h%2)..+64 of
    # qz[h], the other 64 partitions are zero -> scores matmul can use the
    # full 128-partition K^T pair tile as stationary (K=128, stays warm).
    qt_pool = ctx.enter_context(tc.tile_pool(name="qt", bufs=1))
    qz = [qt_pool.tile([P, FREE], BF16, name=f"qz{h}") for h in range(H)]
    for h in range(H):
        off = HD * ((h + 1) % 2)
        nc.vector.memset(qz[h][off:off + HD, :], 0.0)

    # ---- K^T projection, full batch: out[e, s] ----
    load_w((2, 3))
    with tc.tile_pool(name="projk_ps", bufs=8, space="PSUM") as ps_pool:
        for t in range(ET):
            eb = 2 + t // 4
            co = P * (t % 4)
            pss = [ps_pool.tile([P, FREE], F32, name=f"psk{t}_{sch}",
                                tag="proj") for sch in range(4)]
            # d-loop outer: 4 consecutive matmuls share one stationary tile
            for d in range(DCH):
                for sch in range(4):
                    nc.tensor.matmul(pss[sch][:],
                                     wblk[(eb, d)][:, co:co + P],
                                     xt[d][:, FREE * sch:FREE * (sch + 1)],
                                     start=(d == 0), stop=(d == DCH - 1))
            for sch in range(4):
                # cast + K-bias (per-partition) fused on ACT
                nc.scalar.activation(kt[t][:, FREE * sch:FREE * (sch + 1)],
                                     pss[sch][:], AF.Identity,
                                     bias=bqk_sb[:, 8 + t:9 + t])

    # ---- V projection, full batch, natural: out[s, e] ----
    load_w((4, 5))
    with tc.tile_pool(name="projv_ps", bufs=8, space="PSUM") as ps_pool:
        for st in range(NKK):
            pss = [ps_pool.tile([P, FREE], F32, name=f"psv{st}_{eb}",
                                tag="proj") for eb in range(2)]
            for d in range(DCH):
                for eb in range(2):
                    nc.tensor.matmul(pss[eb][:],
                                     xt[d][:, P * st:P * (st + 1)],
                                     wblk[(4 + eb, d)][:],
                                     start=(d == 0), stop=(d == DCH - 1))
            for eb in range(2):
                # cast to the [V|1] attention layout on DVE (no bias:
                # V-bias is folded into the output bias on host)
                v3 = vt[st][:, 0:H * (HD + 1)].rearrange(
                    "p (h c) -> p h c", c=HD + 1)
                nc.vector.tensor_copy(
                    v3[:, 8 * eb:8 * (eb + 1), 0:HD],
                    pss[eb].rearrange("p (h d) -> p h d", d=HD))

    # ---- Q projection (own 512 rows): out[e, q] ----
    load_w((0, 1))
    with tc.tile_pool(name="projq_ps", bufs=4, space="PSUM") as ps_pool:
        for t in range(ET):
            eb = t // 4
            co = P * (t % 4)
            ps = ps_pool.tile([P, FREE], F32, name=f"psq{t}", tag="proj")
            for d in range(DCH):
                nc.tensor.matmul(ps[:], wblk[(eb, d)][:, co:co + P], xq[d][:],
                                 start=(d == 0), stop=(d == DCH - 1))
            nc.scalar.activation(qz[2 * t][0:HD, :], ps[0:HD, :], AF.Identity,
                                 bias=bqk_sb[0:HD, t:t + 1])
            nc.scalar.activation(qz[2 * t + 1][HD:P, :], ps[HD:P, :],
                                 AF.Identity, bias=bqk_sb[HD:P, t:t + 1])

    # ---- prefetch output-projection weights ----
    wo_pool = ctx.enter_context(tc.tile_pool(name="wo", bufs=1))
    wo = []
    for p_ in range(DCH):
        t = wo_pool.tile([P, D], BF16, name=f"wo{p_}")
        nc.sync.dma_start(t[:], woutT.ap()[P * p_:P * (p_ + 1), :])
        wo.append(t)

    # ---- attention, with the output projection interleaved ----
    # outproj partials accumulate into SBUF f32 via DVE, so only one extra
    # PSUM bank is needed and the outproj hides inside attention's ACT slack.
    attn_sb_pool = ctx.enter_context(tc.tile_pool(name="attnsb", bufs=1))
    small_pool = ctx.enter_context(tc.tile_pool(name="small", bufs=4))
    osb_pool = ctx.enter_context(tc.tile_pool(name="osb", bufs=1))
    attn_sb = [attn_sb_pool.tile([P, FREE], BF16, name=f"attnsb{p_}")
               for p_ in range(H // 2)]
    osb = [osb_pool.tile([P, D], F32, name=f"osb{st}")
           for st in range(SL // P)]
    GRP = 2  # kk-chunks per score-psum tile
    with tc.tile_pool(name="sc_ps", bufs=2, space="PSUM") as sc_ps, \
         tc.tile_pool(name="atbc_ps", bufs=2, space="PSUM") as atbc_ps, \
         tc.tile_pool(name="op_ps", bufs=2, space="PSUM") as op_ps, \
         tc.tile_pool(name="e_sb", bufs=4) as e_pool:
        # init osb with the output bias (broadcast via PE outer-product)
        for st in range(SL // P):
            for eb in range(2):
                bi = op_ps.tile([P, FREE], F32, name=f"bi{st}_{eb}", tag="op")
                nc.tensor.matmul(bi[:], ones_bf[:, :P],
                                 bout_sb[:, FREE * eb:FREE * (eb + 1)],
                                 start=True, stop=True)
                nc.vector.tensor_copy(osb[st][:, FREE * eb:FREE * (eb + 1)],
                                      bi[:])

        pending = []
        pv_pending = []
        op_tasks = []

        def normalize(h, at):
            # attn = attnT[0:64] * (1/Z) ; Z = attnT row 64.
            # Broadcast Z first (PE outer-product waits only on the cheap
            # Z-row copy), then reciprocal on DVE off the PE critical path.
            koff = HD * (h % 2)
            rz = small_pool.tile([1, FREE], F32, name=f"rz{h}", tag="rz")
            nc.vector.reciprocal(rz[:], at[HD:HD + 1, :])
            rzb = small_pool.tile([HD, FREE], F32, name=f"rzb{h}", tag="rzb")
            nc.gpsimd.partition_broadcast(rzb[:], rz[:])
            nc.vector.tensor_mul(attn_sb[h // 2][koff:koff + HD, :],
                                 at[0:HD, :], rzb[:])
            # pair h//2 fully normalized once h is odd -> queue its outproj
            if h % 2 == 1:
                p_ = h // 2
                for st in range(SL // P):
                    for eb in range(2):
                        op_tasks.append((p_, st, eb))

        def run_op(p_, st, eb):
            op = op_ps.tile([P, FREE], F32, name=f"op{p_}_{st}_{eb}",
                            tag="op")
            nc.tensor.matmul(op[:], attn_sb[p_][:, P * st:P * (st + 1)],
                             wo[p_][:, FREE * eb:FREE * (eb + 1)],
                             start=True, stop=True)
            nc.vector.tensor_add(osb[st][:, FREE * eb:FREE * (eb + 1)],
                                 osb[st][:, FREE * eb:FREE * (eb + 1)],
                                 op[:])

        def attn_v(h, at, g, e):
            for j in range(GRP):
                kk = GRP * g + j
                nc.tensor.matmul(at[:], vt[kk][:, 65 * h:65 * h + P],
                                 e[:, FREE * j:FREE * (j + 1)],
                                 start=(kk == 0), stop=(kk == NKK - 1))

        for h in range(H):
            ktile = h // 2
            q_rhs = qz[h][:]
            at = atbc_ps.tile([P, FREE], F32, name=f"at{h}", tag="atbc")
            for g in range(NKK // GRP):
                sc = sc_ps.tile([P, GRP * FREE], F32, name=f"sc{h}_{g}",
                                tag="sc")
                for j in range(GRP):
                    kk = GRP * g + j
                    nc.tensor.matmul(
                        sc[:, FREE * j:FREE * (j + 1)],
                        kt[ktile][:, P * kk:P * (kk + 1)],
                        q_rhs, start=True, stop=True)
                e = e_pool.tile([P, GRP * FREE], BF16, name=f"e{h}_{g}",
                                tag="e")
                nc.scalar.activation(e[:], sc[:], AF.Exp)
                # run the PV matmuls one group behind the scores stream, so
                # the PE never waits on the exp it just produced
                if pv_pending:
                    attn_v(*pv_pending.pop())
                pv_pending.append((h, at, g, e))
                if g == 2 and pending:
                    normalize(*pending.pop())
                if op_tasks:
                    run_op(*op_tasks.pop(0))
            pending.append((h, at))
        attn_v(*pv_pending.pop())
        normalize(*pending.pop())
        while op_tasks:
            run_op(*op_tasks.pop(0))
        for st in range(SL // P):
            nc.sync.dma_start(out.ap()[P * st:P * (st + 1), :], osb[st][:])

    ctx.close()


_CACHE = {}


def _get_program():
    if "nc" not in _CACHE:
        _CACHE["nc"] = build_program()
    return _CACHE["nc"]


def prep_inputs(input_tensor, qkv_weight, qkv_bias, out_weight, out_bias):
    """Host-side shard + transpose + cast. Returns in_maps for 8 cores."""
    x = np.asarray(input_tensor, np.float32)
    wqkv = np.asarray(qkv_weight, np.float32).copy()
    bq = np.asarray(qkv_bias, np.float32).copy()
    wout = np.asarray(out_weight, np.float32)
    scale = 1.0 / np.sqrt(np.float32(HD))
    wqkv[:D] *= scale
    bq[:D] *= scale
    bf = ml_dtypes.bfloat16
    wqkvT = np.ascontiguousarray(wqkv.T).astype(bf)
    # Q/K biases, column-major per 128-feature tile: bqk[:, t] = bias tile t
    bqk = np.ascontiguousarray(bq[:2 * D].reshape(16, P).T).astype(bf)
    woutT = np.ascontiguousarray(wout.T).astype(bf)
    # V bias folded into output bias: probs @ (V + b_v) = probs @ V + b_v
    bout_eff = np.asarray(out_bias, np.float32) + wout @ bq[2 * D:]
    bout = bout_eff.reshape(1, D).astype(bf)
    xTb = [np.ascontiguousarray(x[b].T).astype(bf) for b in range(B)]
    in_maps = []
    for c in range(N_CORES):
        b, j = c // R, c % R
        xqT = np.ascontiguousarray(xTb[b][:, SL * j:SL * (j + 1)])
        in_maps.append({"xT": xTb[b], "xqT": xqT, "wqkvT": wqkvT,
                        "bqk": bqk, "woutT": woutT, "bout": bout})
    return in_maps


def kernel(input_tensor, qkv_weight, qkv_bias, out_weight, out_bias,
           **run_kwargs):
    nc = _get_program()
    in_maps = prep_inputs(input_tensor, qkv_weight, qkv_bias, out_weight,
                          out_bias)
    res = run_bass_kernel_spmd(nc, in_maps, core_ids=list(range(N_CORES)),
                               **run_kwargs)
    full = np.empty((B, S, D), np.float32)
    for c in range(N_CORES):
        b, j = c // R, c % R
        full[b, SL * j:SL * (j + 1), :] = res.results[c]["out"]
    if run_kwargs:
        kernel.last_results = res
    return full


# revision 30
# speedup vs baseline: 1.0861x; 1.0206x over previous
"""Multi-head attention (B=2, S=2048, D=1024, H=16) on 8 Trainium2 NeuronCores.

Sharding: core c handles (batch b=c//4, query chunk j=c%4 of 512 rows).
 - Each core computes K^T / V for its WHOLE batch locally (weights replicated,
   pre-transposed + bf16-cast on host; softmax scale folded into W_q) — no
   collectives, the PE stays continuously busy so the HAM clock-gate stays
   warm.
 - Q projected for the core's own 512 rows only.
 - Attention (all 16 heads, 512 queries x 2048 keys):
   scoresT = K_h @ Q_h^T  ->  exp on ACT  ->  attnT = [V_h|1]^T @ E
   (ones column gives the softmax denominator Z in row 64 of attnT psum).
 - Q/K biases folded into the projection casts (ACT Identity per-partition
   bias); V bias folded into the output bias on host (sum(probs) == 1).
 - Output projection local per 512-row chunk; final output assembled on host.
"""

import numpy as np
import ml_dtypes

import concourse.bass as bass
import concourse.mybir as mybir
import concourse.tile as tile
from concourse import bacc
from concourse.bass_utils import run_bass_kernel_spmd

BF16 = mybir.dt.bfloat16
F32 = mybir.dt.float32
AF = mybir.ActivationFunctionType

B, S, D = 2, 2048, 1024
H, HD = 16, 64
N_CORES = 8
R = 4            # cores per batch
SL = S // R      # local query rows per core (512)
P = 128
DCH = D // P     # 8 d-chunks
NKK = S // P     # 16 key chunks
ET = D // P      # 8 feature tiles per projection
FREE = 512


def build_program():
    nc = bacc.Bacc("TRN2", target_bir_lowering=False, debug=False,
                   num_devices=N_CORES)

    xT = nc.dram_tensor("xT", [D, S], BF16, kind="ExternalInput")
    xqT = nc.dram_tensor("xqT", [D, SL], BF16, kind="ExternalInput")
    wqkvT = nc.dram_tensor("wqkvT", [D, 3 * D], BF16, kind="ExternalInput")
    bqk = nc.dram_tensor("bqk", [P, 16], BF16, kind="ExternalInput")
    woutT = nc.dram_tensor("woutT", [D, D], BF16, kind="ExternalInput")
    bout = nc.dram_tensor("bout", [1, D], BF16, kind="ExternalInput")
    out = nc.dram_tensor("out", [SL, D], F32, kind="ExternalOutput")

    with tile.TileContext(nc) as tc:
        _build(nc, tc, xT, xqT, wqkvT, bqk, woutT, bout, out)
    nc.compile()
    return nc


def _build(nc, tc, xT, xqT, wqkvT, bqk, woutT, bout, out):
    from contextlib import ExitStack

    ctx = ExitStack()
    consts = ctx.enter_context(tc.tile_pool(name="consts", bufs=1))

    # ---- constants ----
    ones_bf = consts.tile([1, FREE], BF16, name="ones_bf")
    nc.vector.memset(ones_bf[:], 1.0)
    bqk_sb = consts.tile([P, 16], BF16, name="bqk_sb")
    nc.sync.dma_start(bqk_sb[:], bqk.ap())
    bout_sb = consts.tile([1, D], BF16, name="bout_sb")
    nc.sync.dma_start(bout_sb[:], bout.ap())

    # ---- resident input tiles ----
    xt_pool = ctx.enter_context(tc.tile_pool(name="xt", bufs=1))
    xt = []
    for i in range(DCH):
        t = xt_pool.tile([P, S], BF16, name=f"xt{i}")
        for ch in range(4):
            nc.sync.dma_start(t[:, FREE * ch:FREE * (ch + 1)],
                              xT.ap()[P * i:P * (i + 1),
                                      FREE * ch:FREE * (ch + 1)])
        xt.append(t)
    xq = []
    for i in range(DCH):
        t = xt_pool.tile([P, SL], BF16, name=f"xq{i}")
        nc.sync.dma_start(t[:], xqT.ap()[P * i:P * (i + 1), :])
        xq.append(t)

    # ---- weight stream (K, V blocks first, then Q) ----
    w_pool = ctx.enter_context(tc.tile_pool(name="wq", bufs=16))
    wblk = {}

    def load_w(ebs):
        for eb in ebs:
            for d in range(DCH):
                t = w_pool.tile([P, FREE], BF16, name=f"w{eb}_{d}", tag="w")
                nc.gpsimd.dma_start(t[:], wqkvT.ap()[P * d:P * (d + 1),
                                                     FREE * eb:FREE * (eb + 1)])
                wblk[(eb, d)] = t

    # V layout: per key-tile, 16 heads x (64 V-features + ones col) packed at
    # stride 65, plus 64 zero columns of tail pad so a 128-wide stationary
    # slice [65h : 65h+128] is always in bounds. Columns 65..127 of that
    # slice hit neighbor-head data; they only feed psum rows 65..127 which
    # are never read. Full 128x128 stationary keeps the PE clock-gate warm.
    VW = H * (HD + 1) + HD  # 1104
    kv_pool = ctx.enter_context(tc.tile_pool(name="kv", bufs=1))
    kt = [kv_pool.tile([P, S], BF16, name=f"kt{t}") for t in range(ET)]
    vt = [kv_pool.tile([P, VW], BF16, name=f"vt{g}") for g in range(NKK)]
    for g in range(NKK):
        v3 = vt[g][:, 0:H * (HD + 1)].rearrange("p (h c) -> p h c", c=HD + 1)
        nc.vector.memset(v3[:, :, HD:HD + 1], 1.0)
        nc.vector.memset(vt[g][:, H * (HD + 1):VW], 0.0)
    # Q, zero-padded per head: head h occupies partitions 64*(h%2)..+64 of
    # qz[h], the other 64 partitions are zero -> scores matmul can use the
    # full 128-partition K^T pair tile as stationary (K=128, stays warm).
    qt_pool = ctx.enter_context(tc.tile_pool(name="qt", bufs=1))
    qz = [qt_pool.tile([P, FREE], BF16, name=f"qz{h}") for h in range(H)]
    for h in range(H):
        off = HD * ((h + 1) % 2)
        nc.vector.memset(qz[h][off:off + HD, :], 0.0)

    # ---- K^T projection, full batch: out[e, s] ----
    load_w((2, 3))
    with tc.tile_pool(name="projk_ps", bufs=8, space="PSUM") as ps_pool:
        for t in range(ET):
            eb = 2 + t // 4
            co = P * (t % 4)
            pss = [ps_pool.tile([P, FREE], F32, name=f"psk{t}_{sch}",
                                tag="proj") for sch in range(4)]
            # d-loop outer: 4 consecutive matmuls share one stationary tile
            for d in range(DCH):
                for sch in range(4):
                    nc.tensor.matmul(pss[sch][:],
                                     wblk[(eb, d)][:, co:co + P],
                                     xt[d][:, FREE * sch:FREE * (sch + 1)],
                                     start=(d == 0), stop=(d == DCH - 1))
            for sch in range(4):
                # cast + K-bias (per-partition) fused on ACT
                nc.scalar.activation(kt[t][:, FREE * sch:FREE * (sch + 1)],
                                     pss[sch][:], AF.Identity,
                                     bias=bqk_sb[:, 8 + t:9 + t])

    # ---- V projection, full batch, natural: out[s, e] ----
    load_w((4, 5))
    with tc.tile_pool(name="projv_ps", bufs=8, space="PSUM") as ps_pool:
        for st in range(NKK):
            pss = [ps_pool.tile([P, FREE], F32, name=f"psv{st}_{eb}",
                                tag="proj") for eb in range(2)]
            for d in range(DCH):
                for eb in range(2):
                    nc.tensor.matmul(pss[eb][:],
                                     xt[d][:, P * st:P * (st + 1)],
                                     wblk[(4 + eb, d)][:],
                                     start=(d == 0), stop=(d == DCH - 1))
            for eb in range(2):
                # cast to the [V|1] attention layout on DVE (no bias:
                # V-bias is folded into the output bias on host)
                v3 = vt[st][:, 0:H * (HD + 1)].rearrange(
                    "p (h c) -> p h c", c=HD + 1)
                nc.vector.tensor_copy(
                    v3[:, 8 * eb:8 * (eb + 1), 0:HD],
                    pss[eb].rearrange("p (h d) -> p h d", d=HD))

    # ---- Q projection (own 512 rows): out[e, q] ----
    load_w((0, 1))
    with tc.tile_pool(name="projq_ps", bufs=4, space="PSUM") as ps_pool:
        for t in range(ET):
            eb = t // 4
            co = P * (t % 4)
            ps = ps_pool.tile([P, FREE], F32, name=f"psq{t}", tag="proj")
            for d in range(DCH):
                nc.tensor.matmul(ps[:], wblk[(eb, d)][:, co:co + P], xq[d][:],
                                 start=(d == 0), stop=(d == DCH - 1))
            nc.scalar.activation(qz[2 * t][0:HD, :], ps[0:HD, :], AF.Identity,
                                 bias=bqk_sb[0:HD, t:t + 1])
            nc.scalar.activation(qz[2 * t + 1][HD:P, :], ps[HD:P, :],
                                 AF.Identity, bias=bqk_sb[HD:P, t:t + 1])

    # ---- prefetch output-projection weights ----
    wo_pool = ctx.enter_context(tc.tile_pool(name="wo", bufs=1))
    wo = []
    for p_ in range(DCH):
        t = wo_pool.tile([P, D], BF16, name=f"wo{p_}")
        nc.sync.dma_start(t[:], woutT.ap()[P * p_:P * (p_ + 1), :])
        wo.append(t)

    # ---- attention, with the output projection interleaved ----
    # outproj partials accumulate into SBUF f32 via DVE, so only one extra
    # PSUM bank is needed and the outproj hides inside attention's ACT slack.
    attn_sb_pool = ctx.enter_context(tc.tile_pool(name="attnsb", bufs=1))
    small_pool = ctx.enter_context(tc.tile_pool(name="small", bufs=4))
    osb_pool = ctx.enter_context(tc.tile_pool(name="osb", bufs=1))
    attn_sb = [attn_sb_pool.tile([P, FREE], BF16, name=f"attnsb{p_}")
               for p_ in range(H // 2)]
    osb = [osb_pool.tile([P, D], F32, name=f"osb{st}")
           for st in range(SL // P)]
    GRP = 2  # kk-chunks per score-psum tile
    with tc.tile_pool(name="sc_ps", bufs=2, space="PSUM") as sc_ps, \
         tc.tile_pool(name="atbc_ps", bufs=2, space="PSUM") as atbc_ps, \
         tc.tile_pool(name="op_ps", bufs=2, space="PSUM") as op_ps, \
         tc.tile_pool(name="e_sb", bufs=4) as e_pool:
        # init osb with the output bias (broadcast via PE outer-product)
        for st in range(SL // P):
            for eb in range(2):
                bi = op_ps.tile([P, FREE], F32, name=f"bi{st}_{eb}", tag="op")
                nc.tensor.matmul(bi[:], ones_bf[:, :P],
                                 bout_sb[:, FREE * eb:FREE * (eb + 1)],
                                 start=True, stop=True)
                nc.vector.tensor_copy(osb[st][:, FREE * eb:FREE * (eb + 1)],
                                      bi[:])

        pending = []
        pv_pending = []
        op_tasks = []
        op_stage = []

        def normalize(h, at):
            # attn = attnT[0:64] * (1/Z) ; Z = attnT row 64.
            # Broadcast Z first (PE outer-product waits only on the cheap
            # Z-row copy), then reciprocal on DVE off the PE critical path.
            koff = HD * (h % 2)
            rz = small_pool.tile([1, FREE], F32, name=f"rz{h}", tag="rz")
            nc.vector.reciprocal(rz[:], at[HD:HD + 1, :])
            rzb = small_pool.tile([HD, FREE], F32, name=f"rzb{h}", tag="rzb")
            nc.gpsimd.partition_broadcast(rzb[:], rz[:])
            nc.vector.tensor_mul(attn_sb[h // 2][koff:koff + HD, :],
                                 at[0:HD, :], rzb[:])
            # pair h//2 fully normalized once h is odd -> queue its outproj
            # (delayed one extra head so the DVE recip/mul chain is done)
            if h % 2 == 1:
                p_ = h // 2
                op_tasks.extend(op_stage)
                op_stage.clear()
                for st in range(SL // P):
                    for eb in range(2):
                        op_stage.append((p_, st, eb))

        def run_op(p_, st, eb):
            op = op_ps.tile([P, FREE], F32, name=f"op{p_}_{st}_{eb}",
                            tag="op")
            nc.tensor.matmul(op[:], attn_sb[p_][:, P * st:P * (st + 1)],
                             wo[p_][:, FREE * eb:FREE * (eb + 1)],
                             start=True, stop=True)
            nc.vector.tensor_add(osb[st][:, FREE * eb:FREE * (eb + 1)],
                                 osb[st][:, FREE * eb:FREE * (eb + 1)],
                                 op[:])

        def attn_v(h, at, g, e):
            for j in range(GRP):
                kk = GRP * g + j
                nc.tensor.matmul(at[:], vt[kk][:, 65 * h:65 * h + P],
                                 e[:, FREE * j:FREE * (j + 1)],
                                 start=(kk == 0), stop=(kk == NKK - 1))

        for h in range(H):
            ktile = h // 2
            q_rhs = qz[h][:]
            at = atbc_ps.tile([P, FREE], F32, name=f"at{h}", tag="atbc")
            for g in range(NKK // GRP):
                sc = sc_ps.tile([P, GRP * FREE], F32, name=f"sc{h}_{g}",
                                tag="sc")
                for j in range(GRP):
                    kk = GRP * g + j
                    nc.tensor.matmul(
                        sc[:, FREE * j:FREE * (j + 1)],
                        kt[ktile][:, P * kk:P * (kk + 1)],
                        q_rhs, start=True, stop=True)
                e = e_pool.tile([P, GRP * FREE], BF16, name=f"e{h}_{g}",
                                tag="e")
                nc.scalar.activation(e[:], sc[:], AF.Exp)
                # run the PV matmuls one group behind the scores stream, so
                # the PE never waits on the exp it just produced
                if pv_pending:
                    attn_v(*pv_pending.pop())
                pv_pending.append((h, at, g, e))
                if g == 2 and pending:
                    normalize(*pending.pop())
                if op_tasks:
                    run_op(*op_tasks.pop(0))
            pending.append((h, at))
        attn_v(*pv_pending.pop())
        normalize(*pending.pop())
        op_tasks.extend(op_stage)
        while op_tasks:
            run_op(*op_tasks.pop(0))
        for st in range(SL // P):
            nc.sync.dma_start(out.ap()[P * st:P * (st + 1), :], osb[st][:])

    ctx.close()


_CACHE = {}


def _get_program():
    if "nc" not in _CACHE:
        _CACHE["nc"] = build_program()
    return _CACHE["nc"]


def prep_inputs(input_tensor, qkv_weight, qkv_bias, out_weight, out_bias):
    """Host-side shard + transpose + cast. Returns in_maps for 8 cores."""
    x = np.asarray(input_tensor, np.float32)
    wqkv = np.asarray(qkv_weight, np.float32).copy()
    bq = np.asarray(qkv_bias, np.float32).copy()
    wout = np.asarray(out_weight, np.float32)
    scale = 1.0 / np.sqrt(np.float32(HD))
    wqkv[:D] *= scale
    bq[:D] *= scale
    bf = ml_dtypes.bfloat16
    wqkvT = np.ascontiguousarray(wqkv.T).astype(bf)
    # Q/K biases, column-major per 128-feature tile: bqk[:, t] = bias tile t
    bqk = np.ascontiguousarray(bq[:2 * D].reshape(16, P).T).astype(bf)
    woutT = np.ascontiguousarray(wout.T).astype(bf)
    # V bias folded into output bias: probs @ (V + b_v) = probs @ V + b_v
    bout_eff = np.asarray(out_bias, np.float32) + wout @ bq[2 * D:]
    bout = bout_eff.reshape(1, D).astype(bf)
    xTb = [np.ascontiguousarray(x[b].T).astype(bf) for b in range(B)]
    in_maps = []
    for c in range(N_CORES):
        b, j = c // R, c % R
        xqT = np.ascontiguousarray(xTb[b][:, SL * j:SL * (j + 1)])
        in_maps.append({"xT": xTb[b], "xqT": xqT, "wqkvT": wqkvT,
                        "bqk": bqk, "woutT": woutT, "bout": bout})
    return in_maps


def kernel(input_tensor, qkv_weight, qkv_bias, out_weight, out_bias,
           **run_kwargs):
    nc = _get_program()
    in_maps = prep_inputs(input_tensor, qkv_weight, qkv_bias, out_weight,
                          out_bias)
    res = run_bass_kernel_spmd(nc, in_maps, core_ids=list(range(N_CORES)),
                               **run_kwargs)
    full = np.empty((B, S, D), np.float32)
    for c in range(N_CORES):
        b, j = c // R, c % R
        full[b, SL * j:SL * (j + 1), :] = res.results[c]["out"]
    if run_kwargs:
        kernel.last_results = res
    return full


# revision 32
# speedup vs baseline: 1.1063x; 1.0186x over previous
"""Multi-head attention (B=2, S=2048, D=1024, H=16) on 8 Trainium2 NeuronCores.

Sharding: core c handles (batch b=c//4, query chunk j=c%4 of 512 rows).
 - Each core computes K^T / V for its WHOLE batch locally (weights replicated,
   pre-transposed + bf16-cast on host; softmax scale folded into W_q) — no
   collectives, the PE stays continuously busy so the HAM clock-gate stays
   warm.
 - Q projected for the core's own 512 rows only.
 - Attention (all 16 heads, 512 queries x 2048 keys):
   scoresT = K_h @ Q_h^T  ->  exp on ACT  ->  attnT = [V_h|1]^T @ E
   (ones column gives the softmax denominator Z in row 64 of attnT psum).
 - Q/K biases folded into the projection casts (ACT Identity per-partition
   bias); V bias folded into the output bias on host (sum(probs) == 1).
 - Output projection local per 512-row chunk; final output assembled on host.
"""

import numpy as np
import ml_dtypes

import concourse.bass as bass
import concourse.mybir as mybir
import concourse.tile as tile
from concourse import bacc
from concourse.bass_utils import run_bass_kernel_spmd

BF16 = mybir.dt.bfloat16
F32 = mybir.dt.float32
AF = mybir.ActivationFunctionType

B, S, D = 2, 2048, 1024
H, HD = 16, 64
N_CORES = 8
R = 4            # cores per batch
SL = S // R      # local query rows per core (512)
P = 128
DCH = D // P     # 8 d-chunks
NKK = S // P     # 16 key chunks
ET = D // P      # 8 feature tiles per projection
FREE = 512


def build_program():
    nc = bacc.Bacc("TRN2", target_bir_lowering=False, debug=False,
                   num_devices=N_CORES)

    xT = nc.dram_tensor("xT", [D, S], BF16, kind="ExternalInput")
    xqT = nc.dram_tensor("xqT", [D, SL], BF16, kind="ExternalInput")
    wqkvT = nc.dram_tensor("wqkvT", [D, 3 * D], BF16, kind="ExternalInput")
    bqk = nc.dram_tensor("bqk", [P, 16], BF16, kind="ExternalInput")
    woutT = nc.dram_tensor("woutT", [D, D], BF16, kind="ExternalInput")
    bout = nc.dram_tensor("bout", [1, D], BF16, kind="ExternalInput")
    out = nc.dram_tensor("out", [SL, D], F32, kind="ExternalOutput")

    with tile.TileContext(nc) as tc:
        _build(nc, tc, xT, xqT, wqkvT, bqk, woutT, bout, out)
    nc.compile()
    return nc


def _build(nc, tc, xT, xqT, wqkvT, bqk, woutT, bout, out):
    from contextlib import ExitStack

    ctx = ExitStack()
    consts = ctx.enter_context(tc.tile_pool(name="consts", bufs=1))

    # ---- constants ----
    ones_bf = consts.tile([1, FREE], BF16, name="ones_bf")
    nc.vector.memset(ones_bf[:], 1.0)
    bqk_sb = consts.tile([P, 16], BF16, name="bqk_sb")
    nc.sync.dma_start(bqk_sb[:], bqk.ap())
    bout_sb = consts.tile([1, D], BF16, name="bout_sb")
    nc.sync.dma_start(bout_sb[:], bout.ap())

    # ---- resident input tiles ----
    xt_pool = ctx.enter_context(tc.tile_pool(name="xt", bufs=1))
    xt = []
    for i in range(DCH):
        t = xt_pool.tile([P, S], BF16, name=f"xt{i}")
        for ch in range(4):
            nc.sync.dma_start(t[:, FREE * ch:FREE * (ch + 1)],
                              xT.ap()[P * i:P * (i + 1),
                                      FREE * ch:FREE * (ch + 1)])
        xt.append(t)
    xq = []
    for i in range(DCH):
        t = xt_pool.tile([P, SL], BF16, name=f"xq{i}")
        nc.sync.dma_start(t[:], xqT.ap()[P * i:P * (i + 1), :])
        xq.append(t)

    # ---- weight stream (K, V blocks first, then Q) ----
    w_pool = ctx.enter_context(tc.tile_pool(name="wq", bufs=16))
    wblk = {}

    def load_w(ebs):
        for eb in ebs:
            for d in range(DCH):
                t = w_pool.tile([P, FREE], BF16, name=f"w{eb}_{d}", tag="w")
                nc.gpsimd.dma_start(t[:], wqkvT.ap()[P * d:P * (d + 1),
                                                     FREE * eb:FREE * (eb + 1)])
                wblk[(eb, d)] = t

    # V layout: per key-tile, 16 heads x (64 V-features + ones col) packed at
    # stride 65, plus 64 zero columns of tail pad so a 128-wide stationary
    # slice [65h : 65h+128] is always in bounds. Columns 65..127 of that
    # slice hit neighbor-head data; they only feed psum rows 65..127 which
    # are never read. Full 128x128 stationary keeps the PE clock-gate warm.
    VW = H * (HD + 1) + HD  # 1104
    kv_pool = ctx.enter_context(tc.tile_pool(name="kv", bufs=1))
    kt = [kv_pool.tile([P, S], BF16, name=f"kt{t}") for t in range(ET)]
    vt = [kv_pool.tile([P, VW], BF16, name=f"vt{g}") for g in range(NKK)]
    for g in range(NKK):
        v3 = vt[g][:, 0:H * (HD + 1)].rearrange("p (h c) -> p h c", c=HD + 1)
        nc.vector.memset(v3[:, :, HD:HD + 1], 1.0)
        nc.vector.memset(vt[g][:, H * (HD + 1):VW], 0.0)
    # Q, zero-padded per head: head h occupies partitions 64*(h%2)..+64 of
    # qz[h], the other 64 partitions are zero -> scores matmul can use the
    # full 128-partition K^T pair tile as stationary (K=128, stays warm).
    qt_pool = ctx.enter_context(tc.tile_pool(name="qt", bufs=1))
    qz = [qt_pool.tile([P, FREE], BF16, name=f"qz{h}") for h in range(H)]
    for h in range(H):
        off = HD * ((h + 1) % 2)
        nc.vector.memset(qz[h][off:off + HD, :], 0.0)

    # ---- K^T projection, full batch: out[e, s] ----
    load_w((2, 3))
    with tc.tile_pool(name="projk_ps", bufs=8, space="PSUM") as ps_pool:
        for t in range(ET):
            eb = 2 + t // 4
            co = P * (t % 4)
            pss = [ps_pool.tile([P, FREE], F32, name=f"psk{t}_{sch}",
                                tag="proj") for sch in range(4)]
            # d-loop outer: 4 consecutive matmuls share one stationary tile
            for d in range(DCH):
                for sch in range(4):
                    nc.tensor.matmul(pss[sch][:],
                                     wblk[(eb, d)][:, co:co + P],
                                     xt[d][:, FREE * sch:FREE * (sch + 1)],
                                     start=(d == 0), stop=(d == DCH - 1))
            for sch in range(4):
                # cast + K-bias (per-partition) fused on ACT
                nc.scalar.activation(kt[t][:, FREE * sch:FREE * (sch + 1)],
                                     pss[sch][:], AF.Identity,
                                     bias=bqk_sb[:, 8 + t:9 + t])

    # ---- V projection, full batch, natural: out[s, e] ----
    load_w((4, 5))
    with tc.tile_pool(name="projv_ps", bufs=8, space="PSUM") as ps_pool:
        for st in range(NKK):
            pss = [ps_pool.tile([P, FREE], F32, name=f"psv{st}_{eb}",
                                tag="proj") for eb in range(2)]
            for d in range(DCH):
                for eb in range(2):
                    nc.tensor.matmul(pss[eb][:],
                                     xt[d][:, P * st:P * (st + 1)],
                                     wblk[(4 + eb, d)][:],
                                     start=(d == 0), stop=(d == DCH - 1))
            for eb in range(2):
                # cast to the [V|1] attention layout on DVE (no bias:
                # V-bias is folded into the output bias on host)
                v3 = vt[st][:, 0:H * (HD + 1)].rearrange(
                    "p (h c) -> p h c", c=HD + 1)
                nc.vector.tensor_copy(
                    v3[:, 8 * eb:8 * (eb + 1), 0:HD],
                    pss[eb].rearrange("p (h d) -> p h d", d=HD))

    # ---- Q projection (own 512 rows): out[e, q] ----
    load_w((0, 1))
    with tc.tile_pool(name="projq_ps", bufs=4, space="PSUM") as ps_pool:
        for t in range(ET):
            eb = t // 4
            co = P * (t % 4)
            ps = ps_pool.tile([P, FREE], F32, name=f"psq{t}", tag="proj")
            for d in range(DCH):
                nc.tensor.matmul(ps[:], wblk[(eb, d)][:, co:co + P], xq[d][:],
                                 start=(d == 0), stop=(d == DCH - 1))
            nc.scalar.activation(qz[2 * t][0:HD, :], ps[0:HD, :], AF.Identity,
                                 bias=bqk_sb[0:HD, t:t + 1])
            nc.scalar.activation(qz[2 * t + 1][HD:P, :], ps[HD:P, :],
                                 AF.Identity, bias=bqk_sb[HD:P, t:t + 1])

    # ---- prefetch output-projection weights ----
    wo_pool = ctx.enter_context(tc.tile_pool(name="wo", bufs=1))
    wo = []
    for p_ in range(DCH):
        t = wo_pool.tile([P, D], BF16, name=f"wo{p_}")
        nc.sync.dma_start(t[:], woutT.ap()[P * p_:P * (p_ + 1), :])
        wo.append(t)

    # ---- attention, with the output projection interleaved ----
    # outproj partials accumulate into SBUF f32 via DVE, so only one extra
    # PSUM bank is needed and the outproj hides inside attention's ACT slack.
    attn_sb_pool = ctx.enter_context(tc.tile_pool(name="attnsb", bufs=1))
    small_pool = ctx.enter_context(tc.tile_pool(name="small", bufs=2))
    osb_pool = ctx.enter_context(tc.tile_pool(name="osb", bufs=1))
    attn_sb = [attn_sb_pool.tile([P, FREE], BF16, name=f"attnsb{p_}")
               for p_ in range(H // 2)]
    osb = [osb_pool.tile([P, D], F32, name=f"osb{st}")
           for st in range(SL // P)]
    GRP = 2  # kk-chunks per score-psum tile
    with tc.tile_pool(name="sc_ps", bufs=2, space="PSUM") as sc_ps, \
         tc.tile_pool(name="atbc_ps", bufs=2, space="PSUM") as atbc_ps, \
         tc.tile_pool(name="op_ps", bufs=2, space="PSUM") as op_ps, \
         tc.tile_pool(name="e_sb", bufs=4) as e_pool:
        # init osb with the output bias (broadcast via PE outer-product)
        for st in range(SL // P):
            for eb in range(2):
                bi = op_ps.tile([P, FREE], F32, name=f"bi{st}_{eb}", tag="op")
                nc.tensor.matmul(bi[:], ones_bf[:, :P],
                                 bout_sb[:, FREE * eb:FREE * (eb + 1)],
                                 start=True, stop=True)
                nc.vector.tensor_copy(osb[st][:, FREE * eb:FREE * (eb + 1)],
                                      bi[:])

        pending = []
        pv_pending = []
        op_tasks = []
        op_stage = []

        def normalize(h, at):
            # attn = attnT[0:64] * (1/Z) ; Z = attnT row 64.
            # Broadcast Z first (PE outer-product waits only on the cheap
            # Z-row copy), then reciprocal on DVE off the PE critical path.
            koff = HD * (h % 2)
            # copy attnT out of PSUM first so the at-slot frees immediately
            atsb = small_pool.tile([HD + 1, FREE], F32, name=f"atsb{h}",
                                   tag="atsb")
            nc.vector.tensor_copy(atsb[:], at[0:HD + 1, :])
            rz = small_pool.tile([1, FREE], F32, name=f"rz{h}", tag="rz")
            nc.vector.reciprocal(rz[:], atsb[HD:HD + 1, :])
            rzb = small_pool.tile([HD, FREE], F32, name=f"rzb{h}", tag="rzb")
            nc.gpsimd.partition_broadcast(rzb[:], rz[:])
            nc.vector.tensor_mul(attn_sb[h // 2][koff:koff + HD, :],
                                 atsb[0:HD, :], rzb[:])
            # pair h//2 fully normalized once h is odd -> queue its outproj
            # (delayed one extra head so the DVE recip/mul chain is done)
            if h % 2 == 1:
                p_ = h // 2
                op_tasks.extend(op_stage)
                op_stage.clear()
                for st in range(SL // P):
                    for eb in range(2):
                        op_stage.append((p_, st, eb))

        def run_op(p_, st, eb):
            op = op_ps.tile([P, FREE], F32, name=f"op{p_}_{st}_{eb}",
                            tag="op")
            nc.tensor.matmul(op[:], attn_sb[p_][:, P * st:P * (st + 1)],
                             wo[p_][:, FREE * eb:FREE * (eb + 1)],
                             start=True, stop=True)
            nc.vector.tensor_add(osb[st][:, FREE * eb:FREE * (eb + 1)],
                                 osb[st][:, FREE * eb:FREE * (eb + 1)],
                                 op[:])

        def attn_v(h, at, g, e):
            for j in range(GRP):
                kk = GRP * g + j
                nc.tensor.matmul(at[:], vt[kk][:, 65 * h:65 * h + P],
                                 e[:, FREE * j:FREE * (j + 1)],
                                 start=(kk == 0), stop=(kk == NKK - 1))

        for h in range(H):
            ktile = h // 2
            q_rhs = qz[h][:]
            at = atbc_ps.tile([P, FREE], F32, name=f"at{h}", tag="atbc")
            for g in range(NKK // GRP):
                sc = sc_ps.tile([P, GRP * FREE], F32, name=f"sc{h}_{g}",
                                tag="sc")
                for j in range(GRP):
                    kk = GRP * g + j
                    nc.tensor.matmul(
                        sc[:, FREE * j:FREE * (j + 1)],
                        kt[ktile][:, P * kk:P * (kk + 1)],
                        q_rhs, start=True, stop=True)
                e = e_pool.tile([P, GRP * FREE], BF16, name=f"e{h}_{g}",
                                tag="e")
                nc.scalar.activation(e[:], sc[:], AF.Exp)
                # run the PV matmuls one group behind the scores stream, so
                # the PE never waits on the exp it just produced
                if pv_pending:
                    attn_v(*pv_pending.pop())
                pv_pending.append((h, at, g, e))
                if g == 2 and pending:
                    normalize(*pending.pop())
                if op_tasks:
                    run_op(*op_tasks.pop(0))
            pending.append((h, at))
        attn_v(*pv_pending.pop())
        normalize(*pending.pop())
        op_tasks.extend(op_stage)
        while op_tasks:
            run_op(*op_tasks.pop(0))
        for st in range(SL // P):
            nc.sync.dma_start(out.ap()[P * st:P * (st + 1), :], osb[st][:])

    ctx.close()


_CACHE = {}


def _get_program():
    if "nc" not in _CACHE:
        _CACHE["nc"] = build_program()
    return _CACHE["nc"]


def prep_inputs(input_tensor, qkv_weight, qkv_bias, out_weight, out_bias):
    """Host-side shard + transpose + cast. Returns in_maps for 8 cores."""
    x = np.asarray(input_tensor, np.float32)
    wqkv = np.asarray(qkv_weight, np.float32).copy()
    bq = np.asarray(qkv_bias, np.float32).copy()
    wout = np.asarray(out_weight, np.float32)
    scale = 1.0 / np.sqrt(np.float32(HD))
    wqkv[:D] *= scale
    bq[:D] *= scale
    bf = ml_dtypes.bfloat16
    wqkvT = np.ascontiguousarray(wqkv.T).astype(bf)
    # Q/K biases, column-major per 128-feature tile: bqk[:, t] = bias tile t
    bqk = np.ascontiguousarray(bq[:2 * D].reshape(16, P).T).astype(bf)
    woutT = np.ascontiguousarray(wout.T).astype(bf)
    # V bias folded into output bias: probs @ (V + b_v) = probs @ V + b_v
    bout_eff = np.asarray(out_bias, np.float32) + wout @ bq[2 * D:]
    bout = bout_eff.reshape(1, D).astype(bf)
    xTb = [np.ascontiguousarray(x[b].T).astype(bf) for b in range(B)]
    in_maps = []
    for c in range(N_CORES):
        b, j = c // R, c % R
        xqT = np.ascontiguousarray(xTb[b][:, SL * j:SL * (j + 1)])
        in_maps.append({"xT": xTb[b], "xqT": xqT, "wqkvT": wqkvT,
                        "bqk": bqk, "woutT": woutT, "bout": bout})
    return in_maps


def kernel(input_tensor, qkv_weight, qkv_bias, out_weight, out_bias,
           **run_kwargs):
    nc = _get_program()
    in_maps = prep_inputs(input_tensor, qkv_weight, qkv_bias, out_weight,
                          out_bias)
    res = run_bass_kernel_spmd(nc, in_maps, core_ids=list(range(N_CORES)),
                               **run_kwargs)
    full = np.empty((B, S, D), np.float32)
    for c in range(N_CORES):
        b, j = c // R, c % R
        full[b, SL * j:SL * (j + 1), :] = res.results[c]["out"]
    if run_kwargs:
        kernel.last_results = res
    return full
